# revision 59
# baseline (speedup 1.0000x reference)
"""Boundary rendering module for Trainium2 (8 NeuronCores), single-launch.

Computes, for x of shape (2, 4, 64, 256, 256) f32:
    mn/mx  = per-channel global min/max
    binary = ((x - mn) / (mx - mn)) > 0.5     [== x > (mn + mx)/2]
    dilated = 3x3x3 binary dilation of binary (SAME padding)
    out    = dilated - binary

Sharding: H (=256) split into 8 chunks of 32 rows, one per NeuronCore.
Each core receives its 32 rows plus one halo row on each side (global
edges padded with -1e30 so the halo mask is 0).  On-core layout puts
(B, D) = 128 on the SBUF partition axis; (C, H, W) live on the free axis.

Single NEFF:
  phase 1: SWDGE loads in 8-row (8KB) packets (larger packets fall off
  the SDMA read fast path), DVE min/max reduces interleaved per chunk.
  The cross-core exchange is split into four per-channel mesh AllToAll
  collectives (~43us firmware latency each) so channel c's phase 2
  overlaps channel c+1's exchange.
  phase 2 per (channel, 8-row quarter): threshold on the Scalar engine
  (saturated sigmoid at scale 1e8 -> exact {0,1}), H/W-dilate on DVE,
  D-window count + -16*binary via banded PE matmuls into 2-bank PSUM
  tiles, saturated sigmoid -> out staging, SWDGE stores.
"""

import os
import sys

import numpy as np

for _p in ("/opt/trn_rl_repo", "/root/.axon_site/_ro/trn_rl_repo"):
    if os.path.isdir(_p) and _p not in sys.path:
        sys.path.insert(0, _p)

import ml_dtypes

B, C, D, H, W = 2, 4, 64, 256, 256
NCORES = 8
HS = H // NCORES  # 32 own rows per core
HA = HS + 2  # rows incl halo
HPAD = np.float32(-1e30)  # halo pad at global H edges -> mask 0

# load chunks: strictly <=8KB per-partition packets (9KB packets run 3x
# slower on the SDMA read path); reduce chunks cover own rows 1..32 only.
# channel 0 uses 4-row chunks so its reduction and exchange start ~25us
# earlier (the SWDGE queue round-robins packets of all queued DMAs, so
# chunk completions cluster; finer first chunks complete sooner)
L0 = [(4 * i, 4 * i + 4) for i in range(8)] + [(32, 34)]
R0 = [(1, 4)] + [(4 * i, 4 * i + 4) for i in range(1, 8)] + [(32, 33)]
LROWS = [(0, 8), (8, 16), (16, 24), (24, 32), (32, 34)]
RROWS = [(1, 8), (8, 16), (16, 24), (24, 32), (32, 33)]
CH_LOADS = [L0, LROWS, LROWS, LROWS]
CH_REDS = [R0, RROWS, RROWS, RROWS]
RED_OFF = [0, 9, 14, 19]  # pmax/pmin column offset per channel (9+5+5+5)

_CACHE = {}


def _consts():
    bd = np.arange(128)
    b = bd // D
    d = bd % D
    A = (b[:, None] == b[None, :]) & (np.abs(d[:, None] - d[None, :]) <= 1)
    A = A.astype(ml_dtypes.bfloat16)
    negI = (-16.0 * np.eye(128)).astype(ml_dtypes.bfloat16)
    I128 = np.eye(128, dtype=np.float32)
    return A, negI, I128


# engine balance knobs: quarters whose W-dilation runs as 3 dw-shifted PE
# matmuls (instead of 2 DVE max ops), and quarters whose threshold runs on
# DVE (exact is_gt) instead of the Scalar engine's saturated sigmoid
W_PE = frozenset({1, 2, 5, 9, 11, 14})
T_DVE = frozenset({3, 8, 13})


def _build(variant: str = "full"):
    import concourse.bass as bass
    import concourse.bacc as bacc
    import concourse.mybir as mybir
    import concourse.tile as tile
    from contextlib import ExitStack

    f32 = mybir.dt.float32
    bf16 = mybir.dt.bfloat16
    Alu = mybir.AluOpType
    Act = mybir.ActivationFunctionType

    nc = bacc.Bacc(
        "TRN2",
        target_bir_lowering=False,
        debug=False,
        num_devices=NCORES,
    )

    xs = nc.dram_tensor("xs", [B, C, D, HA, W], f32, kind="ExternalInput")
    out = nc.dram_tensor("out", [B, C, D, HS, W], f32, kind="ExternalOutput")
    A_np, negI_np, I_np = _consts()
    bandA_d = nc.inline_tensor(A_np, name="bandA")
    negI_d = nc.inline_tensor(negI_np, name="negI")
    ident_d = nc.inline_tensor(I_np, name="ident")

    xsa = xs.ap()
    outa = out.ap()

    with ExitStack() as ctx:
        tc = ctx.enter_context(tile.TileContext(nc))
        pers = ctx.enter_context(tc.tile_pool(name="pers", bufs=1))
        binp = ctx.enter_context(tc.tile_pool(name="binp", bufs=6))
        mwp = ctx.enter_context(tc.tile_pool(name="mwp", bufs=3))
        sgp = ctx.enter_context(tc.tile_pool(name="sgp", bufs=4))
        psump = ctx.enter_context(tc.tile_pool(name="psum", bufs=4, space="PSUM"))
        dram = ctx.enter_context(tc.tile_pool(name="dram", bufs=1, space="DRAM"))

        x_all = pers.tile([128, C, HA, W], f32)  # 136 KiB / partition
        # H-dilated mask, triple-buffered manually: rows of 258 with zero
        # pad cols 0 and 257 so the W-shift views read zeros at the edges
        mh0 = pers.tile([128, 8, 258], bf16)
        mh1 = pers.tile([128, 8, 258], bf16)
        mh = [mh0, mh1]
        pmax = pers.tile([128, 24], f32)
        pmin = pers.tile([128, 24], f32)
        red8 = pers.tile([128, 4, 2], f32)  # per channel [mx | -mn] local
        s2 = pers.tile([128, 1], f32)  # per-partition reduced (parts 0..1)
        s16 = pers.tile([128, 8], f32)  # s2 replicated 8x along free axis
        z8 = pers.tile([128, 8], f32)  # zeros
        s1v = pers.tile([128, 72], f32)  # gathered (0:64) + reduced (64:72)
        gv2 = pers.tile([128, 2], f32)  # broadcast [mx | -mn] on all parts
        mnv = pers.tile([128, 4], f32)  # mn per channel
        h4 = pers.tile([128, 4], f32)  # 0.5*(mx-mn) per channel
        bias4 = pers.tile([128, 4], f32)  # -1e8 * (mn + h) per channel
        At = pers.tile([128, 128], bf16)
        Nt = pers.tile([128, 128], bf16)
        It = pers.tile([128, 128], f32)
        ones1 = pers.tile([128, 128], f32)  # row 0 used as all-ones lhsT
        selb = pers.tile([128, 1], f32)

        ccins = [dram.tile([8, 2], f32, name=f"ccin{c}") for c in range(C)]
        ccouts = [dram.tile([8, 2], f32, name=f"ccout{c}") for c in range(C)]
        ccwarm_i = dram.tile([8, 2], f32, name="ccwarm_i")
        ccwarm_o = dram.tile([8, 2], f32, name="ccwarm_o")

        nc.vector.memset(selb[:, :], -100.0)
        nc.vector.memset(ones1[:, :], 1.0)
        nc.vector.memset(z8[:, :], 0.0)
        nc.vector.memset(mh0[:, :, :], 0.0)
        nc.vector.memset(mh1[:, :, :], 0.0)
        nc.sync.dma_start(out=At[:, :], in_=bandA_d.ap())
        nc.sync.dma_start(out=Nt[:, :], in_=negI_d.ap())
        nc.sync.dma_start(out=It[:, :], in_=ident_d.ap())

        # warmup collective: the first collective of a NEFF costs ~68us of
        # firmware warmup; fire a dummy one immediately so it overlaps the
        # loads and the real per-channel exchanges run at ~2-7us
        nc.vector.memset(s1v[:, :], 0.0)
        nc.sync.dma_start(out=ccwarm_i[:, :], in_=z8[0:8, 0:2])
        if variant not in ("nocc", "p2"):
            nc.gpsimd.collective_compute(
                "AllToAll",
                Alu.bypass,
                replica_groups=[list(range(NCORES))],
                ins=[ccwarm_i.opt()],
                outs=[ccwarm_o.opt()],
            )

        # ---- emission helpers ----
        skip_p1 = variant == "p2"

        def emit_loads_reds(c):
            """Channel c: loads, per-chunk min/max reduces, combine, and the
            cross-core exchange trigger."""
            nch = len(CH_LOADS[c])
            for g in range(nch):
                l0, l1 = CH_LOADS[c][g]
                nc.gpsimd.dma_start(
                    out=x_all[:, c, l0:l1, :],
                    in_=xsa[:, c, :, l0:l1, :],
                )
                if skip_p1:
                    continue
                r0, r1 = CH_REDS[c][g]
                k = RED_OFF[c] + g
                chunk = x_all[:, c, r0:r1, :]
                nc.vector.tensor_reduce(
                    out=pmax[:, k : k + 1],
                    in_=chunk,
                    axis=mybir.AxisListType.XY,
                    op=Alu.max,
                )
                nc.vector.tensor_reduce(
                    out=pmin[:, k : k + 1],
                    in_=chunk,
                    axis=mybir.AxisListType.XY,
                    op=Alu.min,
                )
            if skip_p1:
                return
            nc.vector.tensor_reduce(
                out=red8[:, c, 0:1],
                in_=pmax[:, RED_OFF[c] : RED_OFF[c] + nch],
                axis=mybir.AxisListType.X,
                op=Alu.max,
            )
            nc.vector.tensor_reduce(
                out=red8[:, c, 1:2],
                in_=pmin[:, RED_OFF[c] : RED_OFF[c] + nch],
                axis=mybir.AxisListType.X,
                op=Alu.min,
            )
            nc.vector.tensor_scalar_mul(red8[:, c, 1:2], red8[:, c, 1:2], -1.0)
            # cross-partition max via PE transpose of [128p, 2] -> [2p, 128]
            pst = psump.tile([128, 1024], f32, tag="ps")
            nc.tensor.matmul(
                pst[0:2, 0:128], red8[:, c, :], It[:, :], start=True, stop=True
            )
            nc.vector.tensor_reduce(
                out=s2[0:2, 0:1],
                in_=pst[0:2, 0:128],
                axis=mybir.AxisListType.X,
                op=Alu.max,
            )
            nc.vector.tensor_scalar(
                out=s16[0:2, 0:8],
                in0=z8[0:2, 0:8],
                scalar1=s2[0:2, 0:1],
                scalar2=None,
                op0=Alu.add,
            )
            # one-hop mesh AllToAll of this channel's [mx | -mn] pair
            nc.sync.dma_start(
                out=ccins[c][:, :].rearrange("j v -> v j"), in_=s16[0:2, 0:8]
            )
            if variant in ("nocc", "p2"):
                nc.gpsimd.dma_start(out=ccouts[c][:, :], in_=ccins[c][:, :])
            else:
                nc.gpsimd.collective_compute(
                    "AllToAll",
                    Alu.bypass,
                    replica_groups=[list(range(NCORES))],
                    ins=[ccins[c].opt()],
                    outs=[ccouts[c].opt()],
                )

        def emit_bias(c):
            """Channel c: readback of the exchange, global max, rank-1
            broadcast, threshold scalars."""
            nc.sync.dma_start(
                out=s1v[0:1, 16 * c : 16 * c + 16],
                in_=ccouts[c][:, :].rearrange("k v -> (k v)")[None, :],
            )
            nc.vector.tensor_reduce(
                out=s1v[0:1, 64 + 2 * c : 66 + 2 * c],
                in_=s1v[0:1, 16 * c : 16 * c + 16].rearrange(
                    "p (k v) -> p v k", k=NCORES
                ),
                axis=mybir.AxisListType.X,
                op=Alu.max,
            )
            psb = psump.tile([128, 1024], f32, tag="ps")
            nc.tensor.matmul(
                psb[:, 0:2],
                ones1[0:1, :],
                s1v[0:1, 64 + 2 * c : 66 + 2 * c],
                start=True,
                stop=True,
            )
            nc.vector.tensor_copy(gv2[:, :], psb[:, 0:2])
            nc.vector.tensor_scalar_mul(mnv[:, c : c + 1], gv2[:, 1:2], -1.0)
            nc.vector.tensor_add(h4[:, c : c + 1], gv2[:, 0:1], gv2[:, 1:2])
            nc.vector.tensor_scalar_mul(h4[:, c : c + 1], h4[:, c : c + 1], 0.5)
            nc.vector.tensor_add(
                bias4[:, c : c + 1], mnv[:, c : c + 1], h4[:, c : c + 1]
            )
            nc.vector.tensor_scalar_mul(
                bias4[:, c : c + 1], bias4[:, c : c + 1], -1.0e8
            )

        def emit_thresh(idx):
            c, q = idx // 4, idx % 4
            binq = binp.tile([128, 10, W], bf16, tag="binq")
            if idx in T_DVE:
                nc.vector.tensor_scalar(
                    out=binq[:, :, :],
                    in0=x_all[:, c, 8 * q : 8 * q + 10, :],
                    scalar1=mnv[:, c : c + 1],
                    scalar2=h4[:, c : c + 1],
                    op0=Alu.subtract,
                    op1=Alu.is_gt,
                )
            else:
                nc.scalar.activation(
                    out=binq[:, :, :],
                    in_=x_all[:, c, 8 * q : 8 * q + 10, :],
                    func=Act.Sigmoid,
                    bias=bias4[:, c : c + 1],
                    scale=1.0e8,
                )
            return binq

        def emit_rest(idx, binq):
            c, q = idx // 4, idx % 4
            mhq = mh[idx % 2]
            mhd = mhq[:, :, 1:257]
            nc.vector.tensor_tensor(
                out=mhd, in0=binq[:, 0:8, :], in1=binq[:, 2:10, :], op=Alu.max
            )
            nc.vector.tensor_tensor(
                out=mhd, in0=mhd, in1=binq[:, 1:9, :], op=Alu.max
            )
            if idx not in W_PE:
                mwq = mwp.tile([128, 8, W], bf16, tag="mw")
                nc.vector.tensor_tensor(
                    out=mwq[:, :, :],
                    in0=mhq[:, :, 0:256],
                    in1=mhq[:, :, 2:258],
                    op=Alu.max,
                )
                nc.vector.tensor_tensor(
                    out=mwq[:, :, :], in0=mwq[:, :, :], in1=mhd, op=Alu.max
                )
            # sg staged in bf16 ({0,1} exact); the store DMA upcasts to f32,
            # halving SBUF-side store bytes (~1.5x store throughput)
            sg = sgp.tile([128, 2048], bf16, tag="sg")
            for h in range(2):  # two 4-row halves -> 2-bank psum tiles
                ps = psump.tile([128, 1024], f32, tag="ps")
                if idx in W_PE:
                    # W-dilation folded into PE: 3 dw-shifted band matmuls
                    for s in range(2):
                        for j in range(3):
                            nc.tensor.matmul(
                                ps[:, 512 * s : 512 * s + 512],
                                At[:, :],
                                mhq[:, 4 * h + 2 * s : 4 * h + 2 * s + 2, j : j + 256],
                                start=(j == 0),
                                stop=False,
                            )
                else:
                    for s in range(2):
                        nc.tensor.matmul(
                            ps[:, 512 * s : 512 * s + 512],
                            At[:, :],
                            mwq[:, 4 * h + 2 * s : 4 * h + 2 * s + 2, :],
                            start=True,
                            stop=False,
                        )
                for s in range(2):
                    R = 4 * h + 2 * s
                    nc.tensor.matmul(
                        ps[:, 512 * s : 512 * s + 512],
                        Nt[:, :],
                        binq[:, R + 1 : R + 3, :],
                        start=False,
                        stop=True,
                    )
                nc.scalar.activation(
                    out=sg[:, 1024 * h : 1024 * h + 1024],
                    in_=ps[:, :],
                    func=Act.Sigmoid,
                    bias=selb[:, :],
                    scale=200.0,
                )
            nc.gpsimd.dma_start(
                out=outa[:, c, :, 8 * q : 8 * q + 8, :],
                in_=sg.rearrange("p (r w) -> p r w", w=W),
            )

        # ---- interleaved schedule ----
        # Per channel: loads+reduces+exchange trigger; then the PREVIOUS
        # channel's bias plumbing and its 4 phase-2 quarters.  DVE's
        # in-order stream then alternates [reduce block c+1 | dilate block
        # c] with the channel-c exchange hidden under the c+1 reduces, and
        # store issues (gpsimd) land after the next channel's load issues.
        # The threshold runs two quarters ahead (deque) so the Scalar
        # engine's in-order stream never blocks DVE on a fresh binq.
        from collections import deque

        pend = deque()

        def emit_phase2(c):
            emit_bias(c)
            for q in range(4):
                idx = 4 * c + q
                pend.append((idx, emit_thresh(idx)))
                while len(pend) > 2:
                    emit_rest(*pend.popleft())

        if variant == "dbg":
            for c in range(C):
                emit_loads_reds(c)
                if not skip_p1:
                    emit_bias(c)
            nc.sync.dma_start(
                out=outa[:, 0, :, 0, 0:8],
                in_=red8[:, :, :].rearrange("p a b -> p (a b)"),
            )
            nc.sync.dma_start(out=outa[:, 0, :, 1, 0:4], in_=mnv[:, :])
            nc.sync.dma_start(out=outa[:, 0, :, 2, 0:4], in_=h4[:, :])
        else:
            for c in range(C):
                emit_loads_reds(c)
            if skip_p1:
                nc.vector.memset(mnv[:, :], -5.0)
                nc.vector.memset(h4[:, :], 5.0)
                nc.vector.memset(bias4[:, :], 0.0)
            else:
                for c in range(C):
                    emit_bias(c)
            for c in range(C):
                for q in range(4):
                    idx = 4 * c + q
                    pend.append((idx, emit_thresh(idx)))
                    while len(pend) > 2:
                        emit_rest(*pend.popleft())
            while pend:
                emit_rest(*pend.popleft())

    nc.compile()
    return nc


def _get_nc_single():
    if "nc1" not in _CACHE:
        _CACHE["nc1"] = _build()
    return _CACHE["nc1"]


def _make_in_maps(x: np.ndarray):
    in_maps = []
    for k in range(NCORES):
        xs = np.empty((B, C, D, HA, W), np.float32)
        lo = k * HS
        xs[:, :, :, 1 : HS + 1, :] = x[:, :, :, lo : lo + HS, :]
        if k > 0:
            xs[:, :, :, 0, :] = x[:, :, :, lo - 1, :]
        else:
            xs[:, :, :, 0, :] = HPAD
        if k < NCORES - 1:
            xs[:, :, :, HS + 1, :] = x[:, :, :, lo + HS, :]
        else:
            xs[:, :, :, HS + 1, :] = HPAD
        in_maps.append({"xs": xs})
    return in_maps


def kernel(x: np.ndarray) -> np.ndarray:
    from concourse.bass_utils import run_bass_kernel_spmd

    x = np.ascontiguousarray(np.asarray(x), dtype=np.float32)
    assert x.shape == (B, C, D, H, W)

    in_maps = _make_in_maps(x)
    res = run_bass_kernel_spmd(
        _get_nc_single(), in_maps, core_ids=list(range(NCORES))
    )
    pieces = [res.results[k]["out"] for k in range(NCORES)]
    return np.concatenate(pieces, axis=3)


if __name__ == "__main__":
    x = np.random.randn(B, C, D, H, W).astype(np.float32)
    y = kernel(x)
    print(y.shape, y.dtype, y.sum())


# revision 60
# speedup vs baseline: 1.1944x; 1.1944x over previous
"""Boundary rendering module for Trainium2 (8 NeuronCores), single-launch.

Computes, for x of shape (2, 4, 64, 256, 256) f32:
    mn/mx  = per-channel global min/max
    binary = ((x - mn) / (mx - mn)) > 0.5     [== x > (mn + mx)/2]
    dilated = 3x3x3 binary dilation of binary (SAME padding)
    out    = dilated - binary

Sharding: H (=256) split into 8 chunks of 32 rows, one per NeuronCore.
Each core receives its 32 rows plus one halo row on each side (global
edges padded with -1e30 so the halo mask is 0).  On-core layout puts
(B, D) = 128 on the SBUF partition axis; (C, H, W) live on the free axis.

Single NEFF:
  phase 1: SWDGE loads in 8-row (8KB) packets (larger packets fall off
  the SDMA read fast path), DVE min/max reduces interleaved per chunk.
  The cross-core exchange is split into four per-channel mesh AllToAll
  collectives (~43us firmware latency each) so channel c's phase 2
  overlaps channel c+1's exchange.
  phase 2 per (channel, 8-row quarter): threshold on the Scalar engine
  (saturated sigmoid at scale 1e8 -> exact {0,1}), H/W-dilate on DVE,
  D-window count + -16*binary via banded PE matmuls into 2-bank PSUM
  tiles, saturated sigmoid -> out staging, SWDGE stores.
"""

import os
import sys

import numpy as np

for _p in ("/opt/trn_rl_repo", "/root/.axon_site/_ro/trn_rl_repo"):
    if os.path.isdir(_p) and _p not in sys.path:
        sys.path.insert(0, _p)

import ml_dtypes

B, C, D, H, W = 2, 4, 64, 256, 256
NCORES = 8
HS = H // NCORES  # 32 own rows per core
HA = HS + 2  # rows incl halo
HPAD = np.float32(-1e30)  # halo pad at global H edges -> mask 0

# load chunks: strictly <=8KB per-partition packets (9KB packets run 3x
# slower on the SDMA read path); reduce chunks cover own rows 1..32 only.
# channel 0 uses 4-row chunks so its reduction and exchange start ~25us
# earlier (the SWDGE queue round-robins packets of all queued DMAs, so
# chunk completions cluster; finer first chunks complete sooner)
L0 = [(4 * i, 4 * i + 4) for i in range(8)] + [(32, 34)]
R0 = [(1, 4)] + [(4 * i, 4 * i + 4) for i in range(1, 8)] + [(32, 33)]
LROWS = [(0, 8), (8, 16), (16, 24), (24, 32), (32, 34)]
RROWS = [(1, 8), (8, 16), (16, 24), (24, 32), (32, 33)]
CH_LOADS = [L0, LROWS, LROWS, LROWS]
CH_REDS = [R0, RROWS, RROWS, RROWS]
RED_OFF = [0, 9, 14, 19]  # pmax/pmin column offset per channel (9+5+5+5)

_CACHE = {}


def _consts():
    bd = np.arange(128)
    b = bd // D
    d = bd % D
    A = (b[:, None] == b[None, :]) & (np.abs(d[:, None] - d[None, :]) <= 1)
    A = A.astype(ml_dtypes.bfloat16)
    negI = (-16.0 * np.eye(128)).astype(ml_dtypes.bfloat16)
    I128 = np.eye(128, dtype=np.float32)
    return A, negI, I128


# engine balance knobs: quarters whose W-dilation runs as 3 dw-shifted PE
# matmuls (instead of 2 DVE max ops), and quarters whose threshold runs on
# DVE (exact is_gt) instead of the Scalar engine's saturated sigmoid
W_PE = frozenset({1, 2, 5, 9, 11, 14})
T_DVE = frozenset({3, 8, 13})


def _build(variant: str = "full"):
    import concourse.bass as bass
    import concourse.bacc as bacc
    import concourse.mybir as mybir
    import concourse.tile as tile
    from contextlib import ExitStack

    f32 = mybir.dt.float32
    bf16 = mybir.dt.bfloat16
    Alu = mybir.AluOpType
    Act = mybir.ActivationFunctionType

    nc = bacc.Bacc(
        "TRN2",
        target_bir_lowering=False,
        debug=False,
        num_devices=NCORES,
    )

    xs = nc.dram_tensor("xs", [B, C, D, HA, W], f32, kind="ExternalInput")
    out = nc.dram_tensor("out", [B, C, D, HS, W], f32, kind="ExternalOutput")
    A_np, negI_np, I_np = _consts()
    bandA_d = nc.inline_tensor(A_np, name="bandA")
    negI_d = nc.inline_tensor(negI_np, name="negI")
    ident_d = nc.inline_tensor(I_np, name="ident")

    xsa = xs.ap()
    outa = out.ap()

    with ExitStack() as ctx:
        tc = ctx.enter_context(tile.TileContext(nc))
        pers = ctx.enter_context(tc.tile_pool(name="pers", bufs=1))
        binp = ctx.enter_context(tc.tile_pool(name="binp", bufs=6))
        mwp = ctx.enter_context(tc.tile_pool(name="mwp", bufs=2))
        sgp = ctx.enter_context(tc.tile_pool(name="sgp", bufs=4))
        psump = ctx.enter_context(tc.tile_pool(name="psum", bufs=4, space="PSUM"))
        dram = ctx.enter_context(tc.tile_pool(name="dram", bufs=1, space="DRAM"))

        x_all = pers.tile([128, C, HA, W], f32)  # 136 KiB / partition
        # H-dilated mask, triple-buffered manually: rows of 258 with zero
        # pad cols 0 and 257 so the W-shift views read zeros at the edges
        mh0 = pers.tile([128, 8, 258], bf16)
        mh1 = pers.tile([128, 8, 258], bf16)
        mh = [mh0, mh1]
        pmax = pers.tile([128, 24], f32)
        pmin = pers.tile([128, 24], f32)
        red8 = pers.tile([128, 4, 2], f32)  # per channel [mx | -mn] local
        s2 = pers.tile([128, 1], f32)  # per-partition reduced (parts 0..1)
        s16 = pers.tile([128, 8], f32)  # s2 replicated 8x along free axis
        z8 = pers.tile([128, 8], f32)  # zeros
        s1v = pers.tile([128, 72], f32)  # gathered (0:64) + reduced (64:72)
        gv2 = pers.tile([128, 2], f32)  # broadcast [mx | -mn] on all parts
        mnv = pers.tile([128, 4], f32)  # mn per channel
        h4 = pers.tile([128, 4], f32)  # 0.5*(mx-mn) per channel
        bias4 = pers.tile([128, 4], f32)  # -1e8 * (mn + h) per channel
        At = pers.tile([128, 128], bf16)
        Nt = pers.tile([128, 128], bf16)
        It = pers.tile([128, 128], f32)
        ones1 = pers.tile([128, 128], f32)  # row 0 used as all-ones lhsT
        selb = pers.tile([128, 1], f32)

        ccins = [dram.tile([8, 2], f32, name=f"ccin{c}") for c in range(C)]
        ccouts = [dram.tile([8, 2], f32, name=f"ccout{c}") for c in range(C)]
        ccwarm_i = dram.tile([8, 2], f32, name="ccwarm_i")
        ccwarm_o = dram.tile([8, 2], f32, name="ccwarm_o")

        nc.vector.memset(selb[:, :], -100.0)
        nc.vector.memset(ones1[:, :], 1.0)
        nc.vector.memset(z8[:, :], 0.0)
        nc.vector.memset(mh0[:, :, :], 0.0)
        nc.vector.memset(mh1[:, :, :], 0.0)
        nc.sync.dma_start(out=At[:, :], in_=bandA_d.ap())
        nc.sync.dma_start(out=Nt[:, :], in_=negI_d.ap())
        nc.sync.dma_start(out=It[:, :], in_=ident_d.ap())

        # warmup collective: the first collective of a NEFF costs ~68us of
        # firmware warmup; fire a dummy one immediately so it overlaps the
        # loads and the real per-channel exchanges run at ~2-7us
        nc.vector.memset(s1v[:, :], 0.0)
        nc.sync.dma_start(out=ccwarm_i[:, :], in_=z8[0:8, 0:2])
        if variant not in ("nocc", "p2"):
            nc.gpsimd.collective_compute(
                "AllToAll",
                Alu.bypass,
                replica_groups=[list(range(NCORES))],
                ins=[ccwarm_i.opt()],
                outs=[ccwarm_o.opt()],
            )

        # ---- emission helpers ----
        skip_p1 = variant == "p2"

        def emit_loads_reds(c):
            """Channel c: loads, per-chunk min/max reduces, combine, and the
            cross-core exchange trigger."""
            nch = len(CH_LOADS[c])
            for g in range(nch):
                l0, l1 = CH_LOADS[c][g]
                nc.gpsimd.dma_start(
                    out=x_all[:, c, l0:l1, :],
                    in_=xsa[:, c, :, l0:l1, :],
                )
                if skip_p1:
                    continue
                r0, r1 = CH_REDS[c][g]
                k = RED_OFF[c] + g
                chunk = x_all[:, c, r0:r1, :]
                nc.vector.tensor_reduce(
                    out=pmax[:, k : k + 1],
                    in_=chunk,
                    axis=mybir.AxisListType.XY,
                    op=Alu.max,
                )
                nc.vector.tensor_reduce(
                    out=pmin[:, k : k + 1],
                    in_=chunk,
                    axis=mybir.AxisListType.XY,
                    op=Alu.min,
                )
            if skip_p1:
                return
            nc.vector.tensor_reduce(
                out=red8[:, c, 0:1],
                in_=pmax[:, RED_OFF[c] : RED_OFF[c] + nch],
                axis=mybir.AxisListType.X,
                op=Alu.max,
            )
            nc.vector.tensor_reduce(
                out=red8[:, c, 1:2],
                in_=pmin[:, RED_OFF[c] : RED_OFF[c] + nch],
                axis=mybir.AxisListType.X,
                op=Alu.min,
            )
            nc.vector.tensor_scalar_mul(red8[:, c, 1:2], red8[:, c, 1:2], -1.0)
            # cross-partition max via PE transpose of [128p, 2] -> [2p, 128]
            pst = psump.tile([128, 1024], f32, tag="ps")
            nc.tensor.matmul(
                pst[0:2, 0:128], red8[:, c, :], It[:, :], start=True, stop=True
            )
            nc.vector.tensor_reduce(
                out=s2[0:2, 0:1],
                in_=pst[0:2, 0:128],
                axis=mybir.AxisListType.X,
                op=Alu.max,
            )
            nc.vector.tensor_scalar(
                out=s16[0:2, 0:8],
                in0=z8[0:2, 0:8],
                scalar1=s2[0:2, 0:1],
                scalar2=None,
                op0=Alu.add,
            )
            # one-hop mesh AllToAll of this channel's [mx | -mn] pair
            nc.sync.dma_start(
                out=ccins[c][:, :].rearrange("j v -> v j"), in_=s16[0:2, 0:8]
            )
            if variant in ("nocc", "p2"):
                nc.gpsimd.dma_start(out=ccouts[c][:, :], in_=ccins[c][:, :])
            else:
                nc.gpsimd.collective_compute(
                    "AllToAll",
                    Alu.bypass,
                    replica_groups=[list(range(NCORES))],
                    ins=[ccins[c].opt()],
                    outs=[ccouts[c].opt()],
                )

        def emit_bias(c):
            """Channel c: readback of the exchange, global max, rank-1
            broadcast, threshold scalars."""
            nc.sync.dma_start(
                out=s1v[0:1, 16 * c : 16 * c + 16],
                in_=ccouts[c][:, :].rearrange("k v -> (k v)")[None, :],
            )
            nc.vector.tensor_reduce(
                out=s1v[0:1, 64 + 2 * c : 66 + 2 * c],
                in_=s1v[0:1, 16 * c : 16 * c + 16].rearrange(
                    "p (k v) -> p v k", k=NCORES
                ),
                axis=mybir.AxisListType.X,
                op=Alu.max,
            )
            psb = psump.tile([128, 1024], f32, tag="ps")
            nc.tensor.matmul(
                psb[:, 0:2],
                ones1[0:1, :],
                s1v[0:1, 64 + 2 * c : 66 + 2 * c],
                start=True,
                stop=True,
            )
            nc.vector.tensor_copy(gv2[:, :], psb[:, 0:2])
            nc.vector.tensor_scalar_mul(mnv[:, c : c + 1], gv2[:, 1:2], -1.0)
            nc.vector.tensor_add(h4[:, c : c + 1], gv2[:, 0:1], gv2[:, 1:2])
            nc.vector.tensor_scalar_mul(h4[:, c : c + 1], h4[:, c : c + 1], 0.5)
            nc.vector.tensor_add(
                bias4[:, c : c + 1], mnv[:, c : c + 1], h4[:, c : c + 1]
            )
            nc.vector.tensor_scalar_mul(
                bias4[:, c : c + 1], bias4[:, c : c + 1], -1.0e8
            )

        def emit_thresh(idx):
            c, q = idx // 4, idx % 4
            binq = binp.tile([128, 10, W], bf16, tag="binq")
            if idx in T_DVE:
                nc.vector.tensor_scalar(
                    out=binq[:, :, :],
                    in0=x_all[:, c, 8 * q : 8 * q + 10, :],
                    scalar1=mnv[:, c : c + 1],
                    scalar2=h4[:, c : c + 1],
                    op0=Alu.subtract,
                    op1=Alu.is_gt,
                )
            else:
                nc.scalar.activation(
                    out=binq[:, :, :],
                    in_=x_all[:, c, 8 * q : 8 * q + 10, :],
                    func=Act.Sigmoid,
                    bias=bias4[:, c : c + 1],
                    scale=1.0e8,
                )
            return binq

        def emit_rest(idx, binq):
            c, q = idx // 4, idx % 4
            mhq = mh[idx % 2]
            mhd = mhq[:, :, 1:257]
            nc.vector.tensor_tensor(
                out=mhd, in0=binq[:, 0:8, :], in1=binq[:, 2:10, :], op=Alu.max
            )
            nc.vector.tensor_tensor(
                out=mhd, in0=mhd, in1=binq[:, 1:9, :], op=Alu.max
            )
            if idx not in W_PE:
                mwq = mwp.tile([128, 8, W], bf16, tag="mw")
                nc.vector.tensor_tensor(
                    out=mwq[:, :, :],
                    in0=mhq[:, :, 0:256],
                    in1=mhq[:, :, 2:258],
                    op=Alu.max,
                )
                nc.vector.tensor_tensor(
                    out=mwq[:, :, :], in0=mwq[:, :, :], in1=mhd, op=Alu.max
                )
            # sg staged in bf16 ({0,1} exact); the store DMA upcasts to f32,
            # halving SBUF-side store bytes (~1.5x store throughput)
            sg = sgp.tile([128, 2048], bf16, tag="sg")
            for h in range(2):  # two 4-row halves -> 2-bank psum tiles
                ps = psump.tile([128, 1024], f32, tag="ps")
                if idx in W_PE:
                    # W-dilation folded into PE: 3 dw-shifted band matmuls
                    for s in range(2):
                        for j in range(3):
                            nc.tensor.matmul(
                                ps[:, 512 * s : 512 * s + 512],
                                At[:, :],
                                mhq[:, 4 * h + 2 * s : 4 * h + 2 * s + 2, j : j + 256],
                                start=(j == 0),
                                stop=False,
                            )
                else:
                    for s in range(2):
                        nc.tensor.matmul(
                            ps[:, 512 * s : 512 * s + 512],
                            At[:, :],
                            mwq[:, 4 * h + 2 * s : 4 * h + 2 * s + 2, :],
                            start=True,
                            stop=False,
                        )
                for s in range(2):
                    R = 4 * h + 2 * s
                    nc.tensor.matmul(
                        ps[:, 512 * s : 512 * s + 512],
                        Nt[:, :],
                        binq[:, R + 1 : R + 3, :],
                        start=False,
                        stop=True,
                    )
                nc.scalar.activation(
                    out=sg[:, 1024 * h : 1024 * h + 1024],
                    in_=ps[:, :],
                    func=Act.Sigmoid,
                    bias=selb[:, :],
                    scale=200.0,
                )
            nc.gpsimd.dma_start(
                out=outa[:, c, :, 8 * q : 8 * q + 8, :],
                in_=sg.rearrange("p (r w) -> p r w", w=W),
            )

        # ---- interleaved schedule ----
        # Per channel: loads+reduces+exchange trigger; then the PREVIOUS
        # channel's bias plumbing and its 4 phase-2 quarters.  DVE's
        # in-order stream then alternates [reduce block c+1 | dilate block
        # c] with the channel-c exchange hidden under the c+1 reduces, and
        # store issues (gpsimd) land after the next channel's load issues.
        # The threshold runs two quarters ahead (deque) so the Scalar
        # engine's in-order stream never blocks DVE on a fresh binq.
        from collections import deque

        pend = deque()

        def emit_phase2(c):
            emit_bias(c)
            for q in range(4):
                idx = 4 * c + q
                pend.append((idx, emit_thresh(idx)))
                while len(pend) > 2:
                    emit_rest(*pend.popleft())

        if variant == "dbg":
            for c in range(C):
                emit_loads_reds(c)
                if not skip_p1:
                    emit_bias(c)
            nc.sync.dma_start(
                out=outa[:, 0, :, 0, 0:8],
                in_=red8[:, :, :].rearrange("p a b -> p (a b)"),
            )
            nc.sync.dma_start(out=outa[:, 0, :, 1, 0:4], in_=mnv[:, :])
            nc.sync.dma_start(out=outa[:, 0, :, 2, 0:4], in_=h4[:, :])
        else:
            for c in range(C):
                emit_loads_reds(c)
            if skip_p1:
                nc.vector.memset(mnv[:, :], -5.0)
                nc.vector.memset(h4[:, :], 5.0)
                nc.vector.memset(bias4[:, :], 0.0)
            else:
                for c in range(C):
                    emit_bias(c)
            for c in range(C):
                for q in range(4):
                    idx = 4 * c + q
                    pend.append((idx, emit_thresh(idx)))
                    while len(pend) > 2:
                        emit_rest(*pend.popleft())
            while pend:
                emit_rest(*pend.popleft())

    nc.compile()
    return nc


def _get_nc_single():
    if "nc1" not in _CACHE:
        _CACHE["nc1"] = _build()
    return _CACHE["nc1"]


def _make_in_maps(x: np.ndarray):
    in_maps = []
    for k in range(NCORES):
        xs = np.empty((B, C, D, HA, W), np.float32)
        lo = k * HS
        xs[:, :, :, 1 : HS + 1, :] = x[:, :, :, lo : lo + HS, :]
        if k > 0:
            xs[:, :, :, 0, :] = x[:, :, :, lo - 1, :]
        else:
            xs[:, :, :, 0, :] = HPAD
        if k < NCORES - 1:
            xs[:, :, :, HS + 1, :] = x[:, :, :, lo + HS, :]
        else:
            xs[:, :, :, HS + 1, :] = HPAD
        in_maps.append({"xs": xs})
    return in_maps


def kernel(x: np.ndarray) -> np.ndarray:
    from concourse.bass_utils import run_bass_kernel_spmd

    x = np.ascontiguousarray(np.asarray(x), dtype=np.float32)
    assert x.shape == (B, C, D, H, W)

    in_maps = _make_in_maps(x)
    res = run_bass_kernel_spmd(
        _get_nc_single(), in_maps, core_ids=list(range(NCORES))
    )
    pieces = [res.results[k]["out"] for k in range(NCORES)]
    return np.concatenate(pieces, axis=3)


if __name__ == "__main__":
    x = np.random.randn(B, C, D, H, W).astype(np.float32)
    y = kernel(x)
    print(y.shape, y.dtype, y.sum())
